# revision 1
# baseline (speedup 1.0000x reference)
"""Trainium2 Bass kernel for nn_Conv2dP4P4 (P4->P4 group-equivariant conv).

Math (verified vs reference):
  W2 = w.reshape(64,4,64,3,3).sum(1)                  # tap-sum absorbs the
                                                      # reference's group-sum
  out[b, 16q+m', i] = rot90( conv_valid(x[b,:,j], rot90(W2[16q:16q+16], k=i)),
                             k=-i )   with j = (q+i)%4

Per core (8 cores, batch-sharded: 2 batches/core), per unit (b, j):
  - slab S in SBUF [128, H*W] bf16: partitions 0:64 = x[b,:,j],
    partitions 64:128 = x[b,:,j] shifted one row (+W). Taps (0,kw),(1,kw)
    fuse into ONE K=128 matmul; taps (2,kw) are K=64 singles -> 6 matmul
    passes per psum tile instead of 9 (PE is the bottleneck engine).
  - 42 psum tiles [64, 3*126] f32, each = 3 output rows of exactly the
    126 valid columns; kw realized as rhs window offsets (the strided
    3-row rhs view drops the pad lanes). M=64 packs all 4 rotations:
    m-hat blocks: [0:16]=i0, [16:32]=i2, [32:48]=i1, [48:64]=i3.
  - each psum tile evicted ONCE to SBUF E (f32, Act engine); fanout:
    i0 = direct DMA from E (SP queue, zero engine cost); i2 = reversed
    copy into 42-row chunks (DVE); i1/i3 = bf16 fills (scrN on DVE,
    scrR reversed on Pool), PE-transposed two tiles at a time into one
    psum tile, staged by 2 merged Act copies, flushed in 42-column
    v-groups (Act queue).
  - software pipelining: transposes lag >=2 tiles behind the matmuls and
    stages lag one pair more, so PE/Act never wait on the fill chain;
    engine queues chosen so no DMA wait ever blocks a ready dispatch
    (HWDGE dispatch is ~625ns per DMA and the DMA engines serialize:
    count and burst-size of DMAs both matter).
"""
import sys
import numpy as np
import ml_dtypes

sys.path.insert(0, "/opt/trn_rl_repo")

B, C, G, H, W = 16, 64, 4, 128, 128
OUT = 64
HO = H - 2  # 126
NCORES = 8
BPC = B // NCORES  # batches per core
NT = 42   # psum tiles per unit (3 rows each)
RT = 3    # rows per tile
CH = 42   # stage chunk rows

_cache = {}


def _build_weights(w: np.ndarray):
    """w: [256, 64, 3, 3] -> (wt_pair [128, 4j*3kw*64m], wt_sing [64, 4j*3kw*64m])
    bf16, tap-paired: pair rows 0:64 = tap (0,kw), rows 64:128 = tap (1,kw);
    sing = tap (2,kw)."""
    W2 = w.reshape(OUT, 4, C, 3, 3).sum(axis=1)  # [64, 64, 3, 3] f32
    # block order: pos0=i0, pos1=i2, pos2=i1, pos3=i3
    iorder = [0, 2, 1, 3]
    LH = np.zeros((4, 9, C, 64), dtype=np.float32)  # [j, t, c, mhat]
    for j in range(4):
        for pos, i in enumerate(iorder):
            q = (j - i) % 4
            Ki = np.rot90(W2[16 * q:16 * (q + 1)], k=i, axes=(-2, -1))  # [16,64,3,3]
            for kh in range(3):
                for kw in range(3):
                    t = kh * 3 + kw
                    LH[j, t, :, pos * 16:(pos + 1) * 16] = Ki[:, :, kh, kw].T
    wt_pair = np.zeros((128, 4 * 3 * 64), dtype=np.float32)
    wt_sing = np.zeros((64, 4 * 3 * 64), dtype=np.float32)
    for j in range(4):
        for kw in range(3):
            col = (j * 3 + kw) * 64
            wt_pair[0:64, col:col + 64] = LH[j, kw]          # tap (0,kw)
            wt_pair[64:128, col:col + 64] = LH[j, 3 + kw]    # tap (1,kw)
            wt_sing[:, col:col + 64] = LH[j, 6 + kw]         # tap (2,kw)
    return (wt_pair.astype(ml_dtypes.bfloat16),
            wt_sing.astype(ml_dtypes.bfloat16))


def _build_program():
    import concourse.bass as bass
    import concourse.tile as tile
    from concourse import bacc, mybir

    nc = bacc.Bacc("TRN2", target_bir_lowering=False, debug=False)
    x_in = nc.dram_tensor("x_in", [BPC, C, G, H, W], mybir.dt.bfloat16,
                          kind="ExternalInput").ap()
    wp_in = nc.dram_tensor("wp_in", [128, 4 * 3 * 64], mybir.dt.bfloat16,
                           kind="ExternalInput").ap()
    ws_in = nc.dram_tensor("ws_in", [64, 4 * 3 * 64], mybir.dt.bfloat16,
                           kind="ExternalInput").ap()
    eye_in = nc.dram_tensor("eye_in", [128, 128], mybir.dt.bfloat16,
                            kind="ExternalInput").ap()
    o_out = nc.dram_tensor("o_out", [BPC, OUT, 4, HO, HO], mybir.dt.float32,
                           kind="ExternalOutput").ap()

    f32 = mybir.dt.float32
    bf16 = mybir.dt.bfloat16
    HW = H * W

    with tile.TileContext(nc, trace_sim=False) as tc:
        with tc.tile_pool(name="wtp", bufs=1) as wtp, \
             tc.tile_pool(name="slab", bufs=2) as slabp, \
             tc.tile_pool(name="ep", bufs=10) as ep, \
             tc.tile_pool(name="st0", bufs=2) as st0p, \
             tc.tile_pool(name="st2", bufs=2) as st2p, \
             tc.tile_pool(name="stT", bufs=7) as stTp, \
             tc.tile_pool(name="scr", bufs=6) as scrp, \
             tc.tile_pool(name="psc", bufs=6, space="PSUM") as pscp, \
             tc.tile_pool(name="pst", bufs=2, space="PSUM") as pstp:

            wtp2 = wtp.tile([128, 4 * 3 * 64], bf16)
            nc.sync.dma_start(wtp2[:], wp_in)
            wts1 = wtp.tile([64, 4 * 3 * 64], bf16)
            nc.sync.dma_start(wts1[:], ws_in)
            eye = wtp.tile([128, 128], bf16)
            nc.sync.dma_start(eye[:], eye_in)

            def slab_src(u):
                b, j = u // 4, u % 4
                return x_in[b, :, j].rearrange("c h w -> c (h w)")

            def load_slab_piece(S, src, piece):
                # partitions 0:64 = x; partitions 64:128 = x shifted one
                # row (+W); loaded in 4 pieces so transfers interleave with
                # the in-loop output DMAs
                if piece == 0:
                    nc.sync.dma_start(S[0:64, 0:HW // 2], src[:, 0:HW // 2])
                elif piece == 1:
                    nc.sync.dma_start(S[0:64, HW // 2:HW], src[:, HW // 2:])
                elif piece == 2:
                    nc.sync.dma_start(S[64:128, 0:HW // 2],
                                      src[:, W:W + HW // 2])
                else:
                    nc.sync.dma_start(S[64:128, HW // 2:HW - W],
                                      src[:, W + HW // 2:])

            NU = BPC * 4
            carry = []  # deferred prev-unit drain actions
            S_cur = slabp.tile([128, HW + 8], bf16, tag="slab")
            for piece in (0, 2, 1, 3):
                load_slab_piece(S_cur, slab_src(0), piece)

            for u in range(NU):
                b, j = u // 4, u % 4
                S = S_cur
                if u + 1 < NU:
                    S_next = slabp.tile([128, HW + 8], bf16, tag="slab",
                                        name="S_next")
                else:
                    S_next = None

                # output stages: stT1/stT3 split into v-group tiles of GV
                # columns (= GV/6 tile-pairs' stages) so each group flushes
                # independently mid-loop -- no big DMA burst at unit end
                GV = 42  # v-columns per group
                NG = HO // GV  # 3
                t1g, t3g = {}, {}
                g_cnt = {1: [0] * NG, 3: [0] * NG}
                st0_tiles = {}
                st2_tiles = {}

                q0, q2 = j, (j - 2) % 4
                q1, q3 = (j - 1) % 4, (j - 3) % 4

                scr_hist = {}  # k -> (scrN, scrR) awaiting transpose
                pair_ps = {}   # pair p -> (ps, r0) awaiting stage copies

                def tp_pair(p, scr_hist=scr_hist):
                    # 4 transposes (PE) for tile pair (2p, 2p+1) into ONE
                    # psum tile so both tiles' stages merge into 2 copies
                    # (unit state captured via default args: these helpers
                    # may run deferred, inside the NEXT unit's loop)
                    sNa, sRa = scr_hist.pop(2 * p)
                    sNb, sRb = scr_hist.pop(2 * p + 1)
                    ps = pstp.tile([128, 384], bf16, tag="p23")
                    nc.tensor.transpose(ps[0:126, 0:96], sNa[:, 0:126],
                                        eye[0:96, 0:96])
                    nc.tensor.transpose(ps[0:126, 96:192], sRa[:, 0:126],
                                        eye[0:96, 0:96])
                    nc.tensor.transpose(ps[0:126, 192:288], sNb[:, 0:126],
                                        eye[0:96, 0:96])
                    nc.tensor.transpose(ps[0:126, 288:384], sRb[:, 0:126],
                                        eye[0:96, 0:96])
                    return (ps, 6 * p)

                def flushT(rot, g, b=b, q1=q1, q3=q3, t1g=t1g, t3g=t3g):
                    q, tg = (q1, t1g) if rot == 1 else (q3, t3g)
                    nc.scalar.dma_start(
                        o_out[b, 16 * q:16 * (q + 1), rot, :,
                              GV * g:GV * (g + 1)]
                        .rearrange("m u v -> u m v"),
                        tg[g][:].rearrange("p (mm v) -> p mm v", v=GV))

                def stage_pair(ps, r0, t1g=t1g, t3g=t3g, g_cnt=g_cnt,
                               flushT=flushT):
                    # ps blocks: [N_a, R_a, N_b, R_b] at cols 0/96/192/288;
                    # (t s) spans the 6 rows r0..r0+5
                    psr = ps[0:126, :].rearrange("p (t x) -> p t x", t=2)
                    src1 = psr[:, :, 0:96] \
                        .rearrange("p t (s mm) -> p t s mm", s=RT)[:, :, :, 0:16] \
                        .rearrange("p t s mm -> p mm t s")
                    src3 = psr[:, :, 96:192] \
                        .rearrange("p t (s mm) -> p t s mm", s=RT)[:, :, :, 16:32] \
                        .rearrange("p t s mm -> p mm t s")
                    # i1: v-slot = 125 - r_abs -> [120-r0 .. 125-r0] [Act]
                    g1 = (120 - r0) // GV
                    if g1 not in t1g:
                        t1g[g1] = stTp.tile([126, 16 * GV], f32, tag="t1",
                                            name="t1g")
                    lv1 = 120 - r0 - GV * g1
                    nc.scalar.copy(
                        t1g[g1][:].rearrange("p (mm v) -> p mm v", v=GV)
                        [:, :, lv1:lv1 + 6][:, :, ::-1]
                        .rearrange("p mm (t s) -> p mm t s", t=2),
                        src1)
                    g_cnt[1][g1] += 6
                    if g_cnt[1][g1] == GV:
                        flushT(1, g1)
                    # i3: v-slot = r_abs (ascending) [Act]
                    g3 = r0 // GV
                    if g3 not in t3g:
                        t3g[g3] = stTp.tile([126, 16 * GV], f32, tag="t3",
                                            name="t3g")
                    lv3 = r0 - GV * g3
                    nc.scalar.copy(
                        t3g[g3][:].rearrange("p (mm v) -> p mm v", v=GV)
                        [:, :, lv3:lv3 + 6]
                        .rearrange("p mm (t s) -> p mm t s", t=2),
                        src3)
                    g_cnt[3][g3] += 6
                    if g_cnt[3][g3] == GV:
                        flushT(3, g3)

                for k in range(NT):
                    r0 = RT * k  # first output row of this tile
                    # prefetch next unit's slab, one piece every 8 tiles
                    if S_next is not None and k in (8, 16, 24, 32):
                        load_slab_piece(S_next, slab_src(u + 1), k // 8 - 1)
                    pt = pscp.tile([64, RT * 126], f32, tag="conv")
                    pt3 = pt[:].rearrange("m (r x) -> m r x", r=RT)
                    # --- 6 tap matmuls (3 K=128 pairs + 3 K=64 singles);
                    # kw realized as rhs window offsets, psum rows exactly
                    # the 126 valid output columns (no pad lanes)
                    for kw in range(3):
                        base = r0 * W + kw
                        nc.tensor.matmul(
                            pt3[:],
                            wtp2[:, (j * 3 + kw) * 64:(j * 3 + kw) * 64 + 64],
                            S[:, base:base + RT * 128]
                            .rearrange("c (r x) -> c r x", r=RT)[:, :, 0:126],
                            start=(kw == 0), stop=False,
                            skip_group_check=True)
                    for kw in range(3):
                        base = (r0 + 2) * W + kw
                        nc.tensor.matmul(
                            pt3[:],
                            wts1[:, (j * 3 + kw) * 64:(j * 3 + kw) * 64 + 64],
                            S[0:64, base:base + RT * 128]
                            .rearrange("c (r x) -> c r x", r=RT)[:, :, 0:126],
                            start=False, stop=(kw == 2),
                            skip_group_check=True)

                    # --- prev unit's deferred drain: overlaps this
                    # unit's first matmuls instead of stalling PE at the
                    # boundary
                    if carry and k in (1, 2):
                        carry.pop(0)()

                    # --- sw-pipelined: pair transposes lag >=2 tiles,
                    # pair stages one pair later, so neither PE nor Act
                    # ever waits on the fill chain
                    if k >= 3 and k % 2 == 1:
                        p = (k - 3) // 2
                        if p - 1 in pair_ps:
                            stage_pair(*pair_ps.pop(p - 1))
                        pair_ps[p] = tp_pair(p)

                    # --- single eviction psum -> SBUF (Act), then fanout
                    E = ep.tile([64, RT * 126], f32, tag="E")
                    E3 = E[:].rearrange("m (r x) -> m r x", r=RT)
                    nc.scalar.copy(E3, pt3)

                    # --- i0: direct DMA from E (no engine copy) [SP queue]
                    nc.sync.dma_start(
                        o_out[b, 16 * q0:16 * (q0 + 1), 0, r0:r0 + RT],
                        E3[0:16, :, :])

                    # --- T-path fills first (bf16): scrN [DVE], scrR [Pool]
                    # (before i2 so the PE transposes are never gated on i2)
                    scrN = scrp.tile([96, 128], bf16, tag="scrN")
                    scrR = scrp.tile([96, 128], bf16, tag="scrR")
                    for r in range(RT):
                        nc.vector.tensor_copy(
                            scrN[32 * r:32 * r + 32, 0:126],
                            E3[32:64, r, :])
                        nc.gpsimd.tensor_copy(
                            scrR[32 * r:32 * r + 32, 0:126],
                            E3[32:64, r, ::-1])
                    scr_hist[k] = (scrN, scrR)

                    # --- i2: fully reversed copy into chunk [DVE]
                    slot = 125 - (r0 + RT - 1)  # first slot (ascending)
                    c2 = slot // CH
                    if c2 not in st2_tiles:
                        st2_tiles[c2] = st2p.tile([32, CH * HO], f32,
                                                  tag="st2", name="st2c")
                    sl = slot - c2 * CH
                    nc.vector.tensor_copy(
                        st2_tiles[c2][:, sl * HO:(sl + RT) * HO],
                        E3[0:32, ::-1, ::-1])
                    if slot == c2 * CH:  # chunk complete (fills downward)
                        nc.scalar.dma_start(
                            o_out[b, 16 * q2:16 * (q2 + 1), 2,
                                  c2 * CH:c2 * CH + CH]
                            .rearrange("m u v -> m (u v)"),
                            st2_tiles[c2][16:32, :])

                # drain the pipeline (keep <= 2 live ps tiles: pstp
                # bufs=2). Deferred into the next unit's first iterations
                # when one exists, so the drain overlaps its matmuls.
                def _d1(tp_pair=tp_pair, stage_pair=stage_pair,
                        scr_hist=scr_hist, pair_ps=pair_ps):
                    for p in sorted({kk // 2 for kk in scr_hist}):
                        for sp in [s for s in sorted(pair_ps) if s < p]:
                            stage_pair(*pair_ps.pop(sp))
                        pair_ps[p] = tp_pair(p)

                def _d2(stage_pair=stage_pair, pair_ps=pair_ps):
                    for sp in sorted(pair_ps):
                        stage_pair(*pair_ps.pop(sp))

                if S_next is not None:
                    carry = [_d1, _d2]
                else:
                    _d1()
                    _d2()

                S_cur = S_next

    nc.compile()
    return nc


def kernel(x: np.ndarray, w: np.ndarray) -> np.ndarray:
    from concourse.bass_utils import run_bass_kernel_spmd

    if "nc" not in _cache:
        _cache["nc"] = _build_program()
    nc = _cache["nc"]

    wp, ws = _build_weights(np.asarray(w, dtype=np.float32))
    xb = np.ascontiguousarray(
        np.asarray(x, dtype=np.float32).astype(ml_dtypes.bfloat16))
    eye = np.eye(128, dtype=ml_dtypes.bfloat16)
    in_maps = [{"x_in": xb[c * BPC:(c + 1) * BPC], "wp_in": wp, "ws_in": ws,
                "eye_in": eye}
               for c in range(NCORES)]
    _cache["in_maps"] = in_maps
    res = run_bass_kernel_spmd(nc, in_maps, list(range(NCORES)))
    out = np.concatenate([res.results[c]["o_out"] for c in range(NCORES)], axis=0)
    return out.astype(np.float32)



# revision 2
# speedup vs baseline: 1.6594x; 1.6594x over previous
"""Trainium2 Bass kernel for nn_Conv2dP4P4 (P4->P4 group-equivariant conv).

Math (verified vs reference):
  W2 = w.reshape(64,4,64,3,3).sum(1)                  # tap-sum absorbs the
                                                      # reference's group-sum
  out[b, 16q+m', i] = rot90( conv_valid(x[b,:,j], rot90(W2[16q:16q+16], k=i)),
                             k=-i )   with j = (q+i)%4

Device computes ONLY the conv (all 4 rotation blocks packed in M); the final
rot90(k=-i) of each 16-channel block is pure data movement and is applied on
the host after the gather (np.rot90), so the device writes plain m-major
conv-layout tiles at full DMA burst size.

Per core (8 cores, batch-sharded: 2 batches x 4 group elements = 8 units):
  - slab S [128, H*W] bf16: partitions 0:64 = x[b,:,j], 64:128 = x shifted
    one row (+W).
  - 32 psum tiles [128, 504] f32 per unit, RT=4 output rows each (tile 31:
    RT=2 + 3 K=64 singles for the last row's kh=2 taps).  Each tile takes
    THREE K=128, M=128 matmul passes (one per kw):
      M cols   0:64  ("A"): taps (0,kw) via x[a] and (1,kw) via x[a+1]
               -> rows 4k..4k+3
      M cols 64:128  ("B"): tap (2,kw) via x[a+1]  -> rows 4k-1..4k+2
    75% PE utilization vs the 37.5% of a 6-pass M=64 scheme.
  - merge: Act evicts the B half psum[64:128] -> bf16 ring[64:128] (aligned
    copy, one op per tile); DVE then does ONE fused add per tile:
    E[rows 4k..4k+3] = psum_A[0:64, 0:504] + ring[B slots k.s1..s3,(k+1).s0]
    (the ring keeps consecutive tiles' B halves contiguous so the cross-tile
    "tail" row needs no separate op).  TensorTensor allows mixed
    PSUM+SBUF inputs at different base partitions; both-SBUF would not.
  - E [64, 126*126] bf16 staging, flushed in three 42-row DMAs per unit
    (m-major, 10.5KB contiguous runs -> full DMA bandwidth, and the host
    up-casts to f32).
"""
import sys
import numpy as np
import ml_dtypes

sys.path.insert(0, "/opt/trn_rl_repo")

B, C, G, H, W = 16, 64, 4, 128, 128
OUT = 64
HO = H - 2  # 126
NCORES = 8
BPC = B // NCORES  # batches per core
RT = 4
NT = 32            # 31 full tiles + 1 two-row tile
HW = H * W
EPU = HO * HO      # E elements per unit (15876)

_cache = {}


def _build_weights(w: np.ndarray):
    """w: [256, 64, 3, 3] -> (WT [128, 12*128], WTL [64, 12*64]) bf16.

    WT column block (j*3+kw)*128:
      [0:64,   0:64] = LH[j, kw]      (kh0, applied to x[a])
      [64:128, 0:64] = LH[j, 3+kw]    (kh1, applied to x[a+1])
      [64:128,64:128]= LH[j, 6+kw]    (kh2 -> B half, rows a-1)
      [0:64,  64:128]= 0
    WTL column block (j*3+kw)*64 = LH[j, 6+kw] (last-row kh2 singles).
    """
    W2 = w.reshape(OUT, 4, C, 3, 3).sum(axis=1)  # [64, 64, 3, 3] f32
    iorder = [0, 2, 1, 3]
    LH = np.zeros((4, 9, C, 64), dtype=np.float32)  # [j, t, c, mhat]
    for j in range(4):
        for pos, i in enumerate(iorder):
            q = (j - i) % 4
            Ki = np.rot90(W2[16 * q:16 * (q + 1)], k=i, axes=(-2, -1))
            for kh in range(3):
                for kw in range(3):
                    t = kh * 3 + kw
                    LH[j, t, :, pos * 16:(pos + 1) * 16] = Ki[:, :, kh, kw].T
    WT = np.zeros((128, 12 * 128), dtype=np.float32)
    WTL = np.zeros((64, 12 * 64), dtype=np.float32)
    for j in range(4):
        for kw in range(3):
            col = (j * 3 + kw) * 128
            WT[0:64, col:col + 64] = LH[j, kw]
            WT[64:128, col:col + 64] = LH[j, 3 + kw]
            WT[64:128, col + 64:col + 128] = LH[j, 6 + kw]
            WTL[:, (j * 3 + kw) * 64:(j * 3 + kw) * 64 + 64] = LH[j, 6 + kw]
    return (WT.astype(ml_dtypes.bfloat16), WTL.astype(ml_dtypes.bfloat16))


def _build_program():
    import concourse.bass as bass
    import concourse.tile as tile
    from concourse import bacc, mybir

    nc = bacc.Bacc("TRN2", target_bir_lowering=False, debug=False)
    x_in = nc.dram_tensor("x_in", [BPC, C, G, H, W], mybir.dt.bfloat16,
                          kind="ExternalInput").ap()
    wt_in = nc.dram_tensor("wt_in", [128, 12 * 128], mybir.dt.bfloat16,
                           kind="ExternalInput").ap()
    wtl_in = nc.dram_tensor("wtl_in", [64, 12 * 64], mybir.dt.bfloat16,
                            kind="ExternalInput").ap()
    o_out = nc.dram_tensor("o_out", [BPC, G, OUT, HO, HO], mybir.dt.bfloat16,
                           kind="ExternalOutput").ap()

    f32 = mybir.dt.float32
    bf16 = mybir.dt.bfloat16

    with tile.TileContext(nc, trace_sim=False) as tc:
        with tc.tile_pool(name="wtp", bufs=1) as wtp, \
             tc.tile_pool(name="slab", bufs=2) as slabp, \
             tc.tile_pool(name="ep", bufs=2) as ep, \
             tc.tile_pool(name="ring", bufs=1) as ringp, \
             tc.tile_pool(name="psc", bufs=4, space="PSUM") as pscp:

            WT = wtp.tile([128, 12 * 128], bf16)
            nc.sync.dma_start(WT[:], wt_in)
            WTL = wtp.tile([64, 12 * 64], bf16)
            nc.sync.dma_start(WTL[:], wtl_in)
            # B-half ring: one full unit of B evictions, partitions 64:128
            ring = ringp.tile([128, EPU], bf16)

            def slab_src(u):
                b, j = u // 4, u % 4
                return x_in[b, :, j].rearrange("c h w -> c (h w)")

            def load_slab_piece(S, src, piece):
                if piece == 0:
                    nc.sync.dma_start(S[0:64, 0:HW // 2], src[:, 0:HW // 2])
                elif piece == 1:
                    nc.sync.dma_start(S[0:64, HW // 2:HW], src[:, HW // 2:])
                elif piece == 2:
                    nc.sync.dma_start(S[64:128, 0:HW // 2],
                                      src[:, W:W + HW // 2])
                else:
                    nc.sync.dma_start(S[64:128, HW // 2:HW - W],
                                      src[:, W + HW // 2:])

            NU = BPC * 4
            S_cur = slabp.tile([128, HW + 8], bf16, tag="slab")
            for piece in (0, 2, 1, 3):
                load_slab_piece(S_cur, slab_src(0), piece)

            for u in range(NU):
                b, j = u // 4, u % 4
                S = S_cur
                if u + 1 < NU:
                    S_next = slabp.tile([128, HW + 8], bf16, tag="slab",
                                        name="S_next")
                else:
                    S_next = None

                E = ep.tile([64, EPU], bf16, tag="E")
                prev_pt = None

                for k in range(NT):
                    if S_next is not None and k in (6, 12, 18, 24):
                        load_slab_piece(S_next, slab_src(u + 1), k // 6 - 1)

                    pt = pscp.tile([128, RT * 126], f32, tag="conv")
                    if k < NT - 1:
                        p4 = pt[:].rearrange("m (s x) -> m s x", s=RT)
                        for kw in range(3):
                            base = RT * k * W + kw
                            nc.tensor.matmul(
                                p4,
                                WT[:, (j * 3 + kw) * 128:
                                   (j * 3 + kw) * 128 + 128],
                                S[:, base:base + RT * 128]
                                .rearrange("c (s x) -> c s x", s=RT)
                                [:, :, 0:126],
                                start=(kw == 0), stop=(kw == 2),
                                skip_group_check=True)
                        wk = RT * 126
                    else:
                        # rows 124, 125: two windows + kh2 singles for row 125
                        p2 = pt[:, 0:252].rearrange("m (s x) -> m s x", s=2)
                        for kw in range(3):
                            base = RT * k * W + kw
                            nc.tensor.matmul(
                                p2,
                                WT[:, (j * 3 + kw) * 128:
                                   (j * 3 + kw) * 128 + 128],
                                S[:, base:base + 2 * 128]
                                .rearrange("c (s x) -> c s x", s=2)
                                [:, :, 0:126],
                                start=(kw == 0), stop=False,
                                skip_group_check=True)
                        for kw in range(3):
                            base = (H - 1) * W + kw
                            nc.tensor.matmul(
                                pt[0:64, 126:252],
                                WTL[:, (j * 3 + kw) * 64:
                                    (j * 3 + kw) * 64 + 64],
                                S[0:64, base:base + 126],
                                start=False, stop=(kw == 2),
                                skip_group_check=True)
                        wk = 252

                    # Act: evict B half into the ring (aligned partitions)
                    nc.scalar.copy(ring[64:128, 504 * k:504 * k + wk],
                                   pt[64:128, 0:wk])

                    # DVE: fused merge of the PREVIOUS tile (needs this
                    # tile's B slot 0 for its last row)
                    if k >= 1:
                        nc.vector.tensor_add(
                            E[:, 504 * (k - 1):504 * k],
                            prev_pt[0:64, 0:504],
                            ring[64:128, 504 * (k - 1) + 126:
                                 504 * (k - 1) + 630])
                    if k == NT - 1:
                        # row 124 = A s0 + B s1 ; row 125 = A s1 (kh2 singles
                        # already accumulated in psum)
                        nc.vector.tensor_add(
                            E[:, 15624:15750], pt[0:64, 0:126],
                            ring[64:128, 504 * k + 126:504 * k + 252])
                        nc.vector.tensor_copy(E[:, 15750:15876],
                                              pt[0:64, 126:252])
                    prev_pt = pt

                    if k == 11:
                        nc.sync.dma_start(
                            o_out[b, j, :, 0:42, :]
                            .rearrange("m u v -> m (u v)"),
                            E[:, 0:5292])
                    elif k == 21:
                        nc.sync.dma_start(
                            o_out[b, j, :, 42:84, :]
                            .rearrange("m u v -> m (u v)"),
                            E[:, 5292:10584])

                nc.sync.dma_start(
                    o_out[b, j, :, 84:126, :].rearrange("m u v -> m (u v)"),
                    E[:, 10584:15876])

                S_cur = S_next

    nc.compile()
    return nc


def kernel(x: np.ndarray, w: np.ndarray) -> np.ndarray:
    from concourse.bass_utils import run_bass_kernel_spmd

    if "nc" not in _cache:
        _cache["nc"] = _build_program()
    nc = _cache["nc"]

    wt, wtl = _build_weights(np.asarray(w, dtype=np.float32))
    xb = np.ascontiguousarray(
        np.asarray(x, dtype=np.float32).astype(ml_dtypes.bfloat16))
    in_maps = [{"x_in": xb[c * BPC:(c + 1) * BPC], "wt_in": wt,
                "wtl_in": wtl}
               for c in range(NCORES)]
    _cache["in_maps"] = in_maps
    res = run_bass_kernel_spmd(nc, in_maps, list(range(NCORES)))

    iorder = [0, 2, 1, 3]
    out = np.empty((B, OUT, G, HO, HO), dtype=np.float32)
    for c in range(NCORES):
        oc = np.asarray(res.results[c]["o_out"]).astype(np.float32)
        for bi in range(BPC):
            for j in range(4):
                conv = oc[bi, j]  # [64, 126, 126] conv-layout
                for pos, i in enumerate(iorder):
                    q = (j - i) % 4
                    out[c * BPC + bi, 16 * q:16 * (q + 1), i] = np.rot90(
                        conv[16 * pos:16 * (pos + 1)], k=-i, axes=(-2, -1))
    return out


# revision 8
# speedup vs baseline: 1.8690x; 1.1263x over previous
"""Trainium2 Bass kernel for nn_Conv2dP4P4 (P4->P4 group-equivariant conv).

Math (verified vs reference):
  W2 = w.reshape(64,4,64,3,3).sum(1)                  # tap-sum absorbs the
                                                      # reference's group-sum
  out[b, 16q+m', i] = rot90( conv_valid(x[b,:,j], rot90(W2[16q:16q+16], k=i)),
                             k=-i )   with j = (q+i)%4

Device computes ONLY the conv (all 4 rotation blocks packed in M); the final
rot90(k=-i) of each 16-channel block is pure data movement and is applied on
the host after the gather (np.rot90), so the device writes plain m-major
conv-layout tiles at full DMA burst size.

Per core (8 cores, batch-sharded: 2 batches x 4 group elements = 8 units):
  - slab S [128, H*W] bf16: partitions 0:64 = x[b,:,j], 64:128 = x shifted
    one row (+W).
  - 32 psum tiles [128, 504] f32 per unit, RT=4 output rows each (tile 31:
    RT=2 + 3 K=64 singles for the last row's kh=2 taps).  Each tile takes
    THREE K=128, M=128 matmul passes (one per kw):
      M cols   0:64  ("A"): taps (0,kw) via x[a] and (1,kw) via x[a+1]
               -> rows 4k..4k+3
      M cols 64:128  ("B"): tap (2,kw) via x[a+1]  -> rows 4k-1..4k+2
    75% PE utilization vs the 37.5% of a 6-pass M=64 scheme.
  - merge: Act evicts the B half psum[64:128] -> bf16 ring[64:128] (aligned
    copy, one op per tile); DVE then does ONE fused add per tile:
    E[rows 4k..4k+3] = psum_A[0:64, 0:504] + ring[B slots k.s1..s3,(k+1).s0]
    (the ring keeps consecutive tiles' B halves contiguous so the cross-tile
    "tail" row needs no separate op).  TensorTensor allows mixed
    PSUM+SBUF inputs at different base partitions; both-SBUF would not.
  - E [64, 126*126] bf16 staging, flushed in three 42-row DMAs per unit
    (m-major, 10.5KB contiguous runs -> full DMA bandwidth, and the host
    up-casts to f32).
"""
import sys
import numpy as np
import ml_dtypes

sys.path.insert(0, "/opt/trn_rl_repo")

B, C, G, H, W = 16, 64, 4, 128, 128
OUT = 64
HO = H - 2  # 126
NCORES = 8
BPC = B // NCORES  # batches per core
RT = 4
NT = 32            # 31 full tiles + 1 two-row tile
HW = H * W
EPU = HO * HO      # E elements per unit (15876)

_cache = {}


def _build_weights(w: np.ndarray):
    """w: [256, 64, 3, 3] -> (WT [128, 12*128], WTL [64, 12*64]) bf16.

    WT column block (j*3+kw)*128:
      [0:64,   0:64] = LH[j, kw]      (kh0, applied to x[a])
      [64:128, 0:64] = LH[j, 3+kw]    (kh1, applied to x[a+1])
      [64:128,64:128]= LH[j, 6+kw]    (kh2 -> B half, rows a-1)
      [0:64,  64:128]= 0
    WTL column block (j*3+kw)*64 = LH[j, 6+kw] (last-row kh2 singles).
    """
    W2 = w.reshape(OUT, 4, C, 3, 3).sum(axis=1)  # [64, 64, 3, 3] f32
    iorder = [0, 2, 1, 3]
    LH = np.zeros((4, 9, C, 64), dtype=np.float32)  # [j, t, c, mhat]
    for j in range(4):
        for pos, i in enumerate(iorder):
            q = (j - i) % 4
            Ki = np.rot90(W2[16 * q:16 * (q + 1)], k=i, axes=(-2, -1))
            for kh in range(3):
                for kw in range(3):
                    t = kh * 3 + kw
                    LH[j, t, :, pos * 16:(pos + 1) * 16] = Ki[:, :, kh, kw].T
    WT = np.zeros((128, 12 * 128), dtype=np.float32)
    WTL = np.zeros((64, 12 * 64), dtype=np.float32)
    for j in range(4):
        for kw in range(3):
            col = (j * 3 + kw) * 128
            WT[0:64, col:col + 64] = LH[j, kw]
            WT[64:128, col:col + 64] = LH[j, 3 + kw]
            WT[64:128, col + 64:col + 128] = LH[j, 6 + kw]
            WTL[:, (j * 3 + kw) * 64:(j * 3 + kw) * 64 + 64] = LH[j, 6 + kw]
    return (WT.astype(ml_dtypes.bfloat16), WTL.astype(ml_dtypes.bfloat16))


def _build_program():
    import concourse.bass as bass
    import concourse.tile as tile
    from concourse import bacc, mybir

    nc = bacc.Bacc("TRN2", target_bir_lowering=False, debug=False)
    x_in = nc.dram_tensor("x_in", [BPC, C, G, H, W], mybir.dt.bfloat16,
                          kind="ExternalInput").ap()
    wt_in = nc.dram_tensor("wt_in", [128, 12 * 128], mybir.dt.bfloat16,
                           kind="ExternalInput").ap()
    wtl_in = nc.dram_tensor("wtl_in", [64, 12 * 64], mybir.dt.bfloat16,
                            kind="ExternalInput").ap()
    o_out = nc.dram_tensor("o_out", [BPC, G, OUT, HO, HO], mybir.dt.bfloat16,
                           kind="ExternalOutput").ap()

    f32 = mybir.dt.float32
    bf16 = mybir.dt.bfloat16

    with tile.TileContext(nc, trace_sim=False) as tc:
        with tc.tile_pool(name="wtp", bufs=1) as wtp, \
             tc.tile_pool(name="slab", bufs=2) as slabp, \
             tc.tile_pool(name="ep", bufs=2) as ep, \
             tc.tile_pool(name="ring", bufs=1) as ringp, \
             tc.tile_pool(name="psc", bufs=7, space="PSUM") as pscp:

            WT = wtp.tile([128, 12 * 128], bf16)
            nc.sync.dma_start(WT[:], wt_in)
            WTL = wtp.tile([64, 12 * 64], bf16)
            nc.sync.dma_start(WTL[:], wtl_in)
            # B-half ring: one full unit of B evictions, partitions 64:128
            ring = ringp.tile([128, EPU], bf16)

            def slab_src(u):
                b, j = u // 4, u % 4
                return x_in[b, :, j].rearrange("c h w -> c (h w)")

            def load_slab_piece(S, src, piece):
                if piece == 0:
                    nc.sync.dma_start(S[0:64, 0:HW // 2], src[:, 0:HW // 2])
                elif piece == 1:
                    nc.sync.dma_start(S[0:64, HW // 2:HW], src[:, HW // 2:])
                elif piece == 2:
                    nc.sync.dma_start(S[64:128, 0:HW // 2],
                                      src[:, W:W + HW // 2])
                else:
                    nc.sync.dma_start(S[64:128, HW // 2:HW - W],
                                      src[:, W + HW // 2:])

            NU = BPC * 4
            S_cur = slabp.tile([128, HW + 8], bf16, tag="slab")
            # unit 0: fine-grained load so the first matmuls start ~2.7us in
            src0 = slab_src(0)
            for qtr in range(4):
                a, bnd = 4096 * qtr, 4096 * (qtr + 1)
                nc.sync.dma_start(S_cur[0:64, a:bnd], src0[:, a:bnd])
                hi = min(bnd, HW - W)
                nc.sync.dma_start(S_cur[64:128, a:hi], src0[:, W + a:W + hi])

            for u in range(NU):
                b, j = u // 4, u % 4
                S = S_cur
                if u + 1 < NU:
                    S_next = slabp.tile([128, HW + 8], bf16, tag="slab",
                                        name="S_next")
                else:
                    S_next = None

                E = ep.tile([64, EPU], bf16, tag="E")
                prev_pt = None

                for k in range(NT):
                    if S_next is not None and k in (6, 12, 18, 24):
                        load_slab_piece(S_next, slab_src(u + 1), k // 6 - 1)

                    pt = pscp.tile([128, RT * 126], f32, tag="conv")
                    if k < NT - 1:
                        p4 = pt[:].rearrange("m (s x) -> m s x", s=RT)
                        for kw in range(3):
                            base = RT * k * W + kw
                            nc.tensor.matmul(
                                p4,
                                WT[:, (j * 3 + kw) * 128:
                                   (j * 3 + kw) * 128 + 128],
                                S[:, base:base + RT * 128]
                                .rearrange("c (s x) -> c s x", s=RT)
                                [:, :, 0:126],
                                start=(kw == 0), stop=(kw == 2),
                                skip_group_check=True)
                        wk = RT * 126
                    else:
                        # rows 124, 125: two windows + kh2 singles for row 125
                        p2 = pt[:, 0:252].rearrange("m (s x) -> m s x", s=2)
                        for kw in range(3):
                            base = RT * k * W + kw
                            nc.tensor.matmul(
                                p2,
                                WT[:, (j * 3 + kw) * 128:
                                   (j * 3 + kw) * 128 + 128],
                                S[:, base:base + 2 * 128]
                                .rearrange("c (s x) -> c s x", s=2)
                                [:, :, 0:126],
                                start=(kw == 0), stop=False,
                                skip_group_check=True)
                        for kw in range(3):
                            base = (H - 1) * W + kw
                            nc.tensor.matmul(
                                pt[0:64, 126:252],
                                WTL[:, (j * 3 + kw) * 64:
                                    (j * 3 + kw) * 64 + 64],
                                S[0:64, base:base + 126],
                                start=False, stop=(kw == 2),
                                skip_group_check=True)
                        wk = 252

                    # Act: evict B half into the ring (aligned partitions)
                    nc.scalar.copy(ring[64:128, 504 * k:504 * k + wk],
                                   pt[64:128, 0:wk])

                    # DVE: fused merge of the PREVIOUS tile (needs this
                    # tile's B slot 0 for its last row)
                    if k >= 1:
                        nc.vector.tensor_add(
                            E[:, 504 * (k - 1):504 * k],
                            prev_pt[0:64, 0:504],
                            ring[64:128, 504 * (k - 1) + 126:
                                 504 * (k - 1) + 630])
                    if k == NT - 1:
                        # row 124 = A s0 + B s1 ; row 125 = A s1 (kh2 singles
                        # already accumulated in psum)
                        nc.vector.tensor_add(
                            E[:, 15624:15750], pt[0:64, 0:126],
                            ring[64:128, 504 * k + 126:504 * k + 252])
                        nc.scalar.copy(E[:, 15750:15876],
                                       pt[0:64, 126:252])
                    prev_pt = pt

                    if k == 11:
                        nc.sync.dma_start(
                            o_out[b, j, :, 0:42, :]
                            .rearrange("m u v -> m (u v)"),
                            E[:, 0:5292])
                    elif k == 21:
                        nc.sync.dma_start(
                            o_out[b, j, :, 42:84, :]
                            .rearrange("m u v -> m (u v)"),
                            E[:, 5292:10584])
                    elif k == 28:
                        # rows 84..104 ready after merge(26) (tile-27 body)
                        nc.sync.dma_start(
                            o_out[b, j, :, 84:105, :]
                            .rearrange("m u v -> m (u v)"),
                            E[:, 10584:13230])

                nc.sync.dma_start(
                    o_out[b, j, :, 105:126, :]
                    .rearrange("m u v -> m (u v)"),
                    E[:, 13230:15876])

                S_cur = S_next

    nc.compile()
    return nc


def kernel(x: np.ndarray, w: np.ndarray) -> np.ndarray:
    from concourse.bass_utils import run_bass_kernel_spmd

    if "nc" not in _cache:
        _cache["nc"] = _build_program()
    nc = _cache["nc"]

    wt, wtl = _build_weights(np.asarray(w, dtype=np.float32))
    xb = np.ascontiguousarray(
        np.asarray(x, dtype=np.float32).astype(ml_dtypes.bfloat16))
    in_maps = [{"x_in": xb[c * BPC:(c + 1) * BPC], "wt_in": wt,
                "wtl_in": wtl}
               for c in range(NCORES)]
    _cache["in_maps"] = in_maps
    res = run_bass_kernel_spmd(nc, in_maps, list(range(NCORES)))

    iorder = [0, 2, 1, 3]
    out = np.empty((B, OUT, G, HO, HO), dtype=np.float32)
    for c in range(NCORES):
        oc = np.asarray(res.results[c]["o_out"]).astype(np.float32)
        for bi in range(BPC):
            for j in range(4):
                conv = oc[bi, j]  # [64, 126, 126] conv-layout
                for pos, i in enumerate(iorder):
                    q = (j - i) % 4
                    out[c * BPC + bi, 16 * q:16 * (q + 1), i] = np.rot90(
                        conv[16 * pos:16 * (pos + 1)], k=-i, axes=(-2, -1))
    return out
